# revision 1
# baseline (speedup 1.0000x reference)
"""Trainium2 Bass kernel for CrossModalityPositionAttention.

Model (per batch element b of 4):
  q = ConvBNReLU(feature2[b]; qw)   [64, 64, 64]
  k = ConvBNReLU(feature1[b]; kw)
  v = ConvBNReLU(feature1[b]; vw)
  attn = softmax(q^T k over channels), f = v @ attn^T
  out = feature1[b] + ConvBNReLU(f; rw)   [256, 64, 64]

Sharding: 8 cores = 4 batches x 2 spatial halves. Each core computes 34
attention rows (its 32 output rows + 1-row halo on each side; out-of-image
halo rows are masked to zero) against all 4096 key positions, then the
final conv + residual for its own 32 rows. No cross-core communication.

Numerics: convs and score matmuls run in float32r (e8m11, ~2^-12 rel);
attention probabilities and the attn@v matmul run in bf16. Softmax uses a
shifted-exp with a per-row shift alpha[n] = max(S[n, ::8]) + 45 (sampled
row max + margin; margins validated against the reference data), with the
shift injected as an extra contraction channel (k row of ones, q row of
-alpha) so exp(S - alpha) reads straight out of PSUM. A row of ones
appended to v^T makes the same matmul accumulate sum(exp) for the final
normalization.
"""

import sys

sys.path.insert(0, "/opt/trn_rl_repo")

import numpy as np

import concourse.bacc as bacc
import concourse.mybir as mybir
from concourse import tile
from concourse.bass_utils import run_bass_kernel_spmd

F32R = mybir.dt.float32r
F32 = mybir.dt.float32
BF16 = mybir.dt.bfloat16
AF = mybir.ActivationFunctionType
ALU = mybir.AluOpType

EPS = 1e-5
ALPHA_MARGIN = 45.0
H = W = 64
CIN = 256
CMID = 64
NROWS = 34                # attention rows per core (32 + 2 halo)
NLOC = NROWS * W          # 2176
NK = H * W                # 4096 key positions
CHUNK_ROWS = [7, 7, 7, 7, 6]        # attention n-chunks (x64 cols)
MTILES = NK // 128        # 32
REPEAT = 1                # repeat body for differential timing


def rne12(x: np.ndarray) -> np.ndarray:
    """Round fp32 to float32r (e8m11) with round-to-nearest-even."""
    u = np.ascontiguousarray(x, dtype=np.float32).view(np.uint32)
    u = (u + (((u >> 12) & 1) + np.uint32(0x7FF))) & ~np.uint32(0xFFF)
    return u.view(np.float32)


def _build_program(dump=False):
    nc = bacc.Bacc("TRN2", target_bir_lowering=False, debug=False)

    x1_d = nc.dram_tensor("x1", [128, 2, 66, 66], F32R, kind="ExternalInput")
    x2_d = nc.dram_tensor("x2", [128, 2, 36, 66], F32R, kind="ExternalInput")
    wq_d = nc.dram_tensor("wq", [128, 9, 2, 64], F32R, kind="ExternalInput")
    wkv_d = nc.dram_tensor("wkv", [128, 9, 2, 128], F32R, kind="ExternalInput")
    wr_d = nc.dram_tensor("wr", [64, 9, 256], F32R, kind="ExternalInput")
    bn_d = nc.dram_tensor("bn", [128, 10], F32, kind="ExternalInput")
    res_d = nc.dram_tensor("res", [128, 2, 2048], F32, kind="ExternalInput")
    mask_d = nc.dram_tensor("mask", [1, NLOC], F32, kind="ExternalInput")
    out_d = nc.dram_tensor("out", [128, 2, 2048], F32, kind="ExternalOutput")
    if dump:
        dq_d = nc.dram_tensor("d_q", [65, NLOC], F32, kind="ExternalOutput")
        dk_d = nc.dram_tensor("d_k", [65, NK], F32, kind="ExternalOutput")
        dv_d = nc.dram_tensor("d_v", [128, NK], BF16, kind="ExternalOutput")
        dm_d = nc.dram_tensor("d_mcol", [128, 17], F32, kind="ExternalOutput")
        de_d = nc.dram_tensor("d_e0", [128, NLOC], BF16, kind="ExternalOutput")
        df_d = nc.dram_tensor("d_f", [65, NLOC], F32, kind="ExternalOutput")
        dfp_d = nc.dram_tensor("d_fpad", [64, NROWS * 66], F32, kind="ExternalOutput")
        dvt_d = nc.dram_tensor("d_vt", [128, MTILES * 65], BF16, kind="ExternalOutput")

    with tile.TileContext(nc) as tc:
        with tc.tile_pool(name="per", bufs=1) as per, \
             tc.tile_pool(name="eb", bufs=4) as eb, \
             tc.tile_pool(name="sm", bufs=2) as sm, \
             tc.tile_pool(name="tp", bufs=3, space="PSUM") as tp, \
             tc.tile_pool(name="fp", bufs=1, space="PSUM") as fp:

            # ---- persistent SBUF tiles ----
            x1 = per.tile([128, 2, 66, 66], F32R)
            x2 = per.tile([128, 2, 36, 66], F32R)
            wq = per.tile([128, 9, 2, 64], F32R)
            wkv = per.tile([128, 9, 2, 128], F32R)
            wr = per.tile([64, 9, 256], F32R)
            bn = per.tile([128, 10], F32)
            res = per.tile([128, 2, 2048], F32)
            maskrow = per.tile([1, NLOC], F32)
            q_aug = per.tile([65, NLOC], F32R)
            k_aug = per.tile([65, NK], F32R)
            v_bf = per.tile([128, NK], BF16)   # v lives at partitions 64..127
            vT = per.tile([128, MTILES, 80], BF16)  # 80: 32B-aligned tile stride for DMA-transpose dests
            f_pad = per.tile([64, NROWS, 66], F32R)
            mcol = per.tile([128, 17], F32)
            nacol = per.tile([128, 17], F32)
            na_f32 = per.tile([1, NLOC], F32)
            out_sb = per.tile([128, 2, 2048], F32)

            nc.sync.dma_start(out=wkv[:, :, :, :], in_=wkv_d[:, :, :, :])
            nc.sync.dma_start(out=wq[:, :, :, :], in_=wq_d[:, :, :, :])
            nc.sync.dma_start(out=bn[:, :], in_=bn_d[:, :])
            # x1/x2 in row slabs so the first conv tiles start early
            for half in range(2):
                nc.sync.dma_start(out=x1[:, half, 0:19, :], in_=x1_d[:, half, 0:19, :])
                nc.sync.dma_start(out=x1[:, half, 19:35, :],
                                  in_=x1_d[:, half, 19:35, :])
                nc.sync.dma_start(out=x1[:, half, 35:50, :],
                                  in_=x1_d[:, half, 35:50, :])
                nc.sync.dma_start(out=x1[:, half, 50:66, :],
                                  in_=x1_d[:, half, 50:66, :])
            for half in range(2):
                nc.sync.dma_start(out=x2[:, half, 0:18, :], in_=x2_d[:, half, 0:18, :])
                nc.sync.dma_start(out=x2[:, half, 18:36, :],
                                  in_=x2_d[:, half, 18:36, :])
            nc.sync.dma_start(out=maskrow[:, :], in_=mask_d[:, :])
            nc.sync.dma_start(out=wr[:, :, :], in_=wr_d[:, :, :])
            nc.sync.dma_start(out=res[:, :, :], in_=res_d[:, :, :])

            nc.vector.memset(k_aug[64:65, :].bitcast(F32), 1.0)
            nc.vector.memset(vT[:, :, 64:65], 1.0)
            nc.vector.memset(f_pad[:, :, :].bitcast(F32), 0.0)

            # ---- fused k+v conv (M=128: co 0..63 = k, 64..127 = v) ----
            for t in range(8):
                r0 = t * 8
                ps = tp.tile([128, 512], F32, name=f"kv_{t}", tag="tpsum")
                for half in range(2):
                    for off in range(9):
                        dy, dx = off // 3, off % 3
                        nc.tensor.matmul(
                            ps[:, :], wkv[:, off, half, :],
                            x1[:, half, r0 + dy:r0 + dy + 8, dx:dx + W],
                            start=(half == 0 and off == 0),
                            stop=(half == 1 and off == 8))
                nc.scalar.activation(k_aug[0:64, r0 * W:(r0 + 8) * W], ps[0:64, :],
                                     AF.Relu, bias=bn[0:64, 3:4], scale=bn[0:64, 2:3])
                nc.scalar.activation(v_bf[64:128, r0 * W:(r0 + 8) * W], ps[64:128, :],
                                     AF.Relu, bias=bn[64:128, 3:4],
                                     scale=bn[64:128, 2:3])
                # v^T for this 512-col span (4 m-tiles) via DMA transpose
                for mt in range(t * 4, t * 4 + 4):
                    nc.sync.dma_start(out=vT[:, mt, 0:64],
                                      in_=v_bf[64:128, mt * 128:(mt + 1) * 128],
                                      transpose=True)

            # ---- q conv (M=64) ----
            r0 = 0
            sub_done = []
            for t, rows in enumerate(CHUNK_ROWS):
                na = rows * W
                ps = tp.tile([128, 512], F32, name=f"qc_{t}", tag="tpsum")
                for half in range(2):
                    for off in range(9):
                        dy, dx = off // 3, off % 3
                        nc.tensor.matmul(
                            ps[0:64, 0:na], wq[:, off, half, :],
                            x2[:, half, r0 + dy:r0 + dy + rows, dx:dx + W],
                            start=(half == 0 and off == 0),
                            stop=(half == 1 and off == 8))
                nc.scalar.activation(q_aug[0:64, r0 * W:r0 * W + na], ps[0:64, 0:na],
                                     AF.Relu, bias=bn[0:64, 1:2], scale=bn[0:64, 0:1])
                r0 += rows
                # sampled row-max S_sub tiles whose q columns are now ready
                while len(sub_done) < 17 and (len(sub_done) + 1) * 128 <= r0 * W:
                    st_ = len(sub_done)
                    sps = tp.tile([128, 512], F32, name=f"sub_{st_}", tag="tpsum")
                    nc.tensor.matmul(sps[:, :],
                                     q_aug[0:64, st_ * 128:(st_ + 1) * 128],
                                     k_aug[0:64, ::8], start=True, stop=True)
                    nc.vector.tensor_reduce(mcol[:, st_:st_ + 1], sps[:, :],
                                            axis=mybir.AxisListType.X, op=ALU.max)
                    sub_done.append(st_)

            # -alpha = -(submax + MARGIN)
            nc.vector.tensor_scalar(nacol[:, :], mcol[:, :], -1.0, -ALPHA_MARGIN,
                                    ALU.mult, ALU.add)
            for t in range(17):
                nc.sync.dma_start(out=na_f32[:, t * 128:(t + 1) * 128],
                                  in_=nacol[:, t:t + 1])
            nc.vector.tensor_copy(q_aug[64:65, :], na_f32[:, :])

            # ---- attention: S^T -> exp -> attn @ v (+ sumexp row) ----
            cstart = [0]
            for r in CHUNK_ROWS:
                cstart.append(cstart[-1] + r * W)
            if dump:
                de_sb = per.tile([128, NLOC], BF16)
            fbanks = [fp.tile([65, CHUNK_ROWS[c] * W], F32, name=f"fb{c}")
                      for c in range(5)]
            for m in range(MTILES):
                for c in range(5):
                    n0, n1 = cstart[c], cstart[c + 1]
                    st = tp.tile([128, 512], F32, name=f"st_{m}_{c}", tag="tpsum")
                    nc.tensor.matmul(st[:, 0:n1 - n0], k_aug[:, m * 128:(m + 1) * 128],
                                     q_aug[:, n0:n1], start=True, stop=True)
                    e = eb.tile([128, 512], BF16, name=f"e_{m}_{c}", tag="ebuf")
                    nc.scalar.activation(e[:, 0:n1 - n0], st[:, 0:n1 - n0], AF.Exp)
                    nc.tensor.matmul(fbanks[c][:, :], vT[:, m, 0:65], e[:, 0:n1 - n0],
                                     start=(m == 0), stop=(m == MTILES - 1))
                    if dump and m == 0:
                        nc.vector.tensor_copy(de_sb[:, n0:n1], e[:, 0:n1 - n0])

            # ---- normalize f and store into padded layout ----
            for c in range(5):
                n0, n1 = cstart[c], cstart[c + 1]
                rcp = sm.tile([1, 512], F32, name=f"rcp{c}", tag="rcp")
                nc.vector.reciprocal(rcp[:, 0:n1 - n0], fbanks[c][64:65, :])
                rcpm = sm.tile([1, 512], F32, name=f"rcpm{c}", tag="rcpm")
                nc.vector.tensor_tensor(rcpm[:, 0:n1 - n0], rcp[:, 0:n1 - n0],
                                        maskrow[:, n0:n1], op=ALU.mult)
                rb = sm.tile([64, 512], F32, name=f"rb{c}", tag="rb")
                nc.gpsimd.partition_broadcast(rb[:, 0:n1 - n0], rcpm[:, 0:n1 - n0])
                row0 = n0 // W
                nc.vector.tensor_tensor(
                    f_pad[:, row0:row0 + CHUNK_ROWS[c], 1:65],
                    fbanks[c][0:64, :], rb[:, 0:n1 - n0], op=ALU.mult)

            # ---- final conv(64->256) + BN + ReLU + residual ----
            for coh in range(2):
                for t in range(4):
                    ps = tp.tile([128, 512], F32, name=f"rps_{coh}_{t}", tag="tpsum")
                    for off in range(9):
                        dy, dx = off // 3, off % 3
                        nc.tensor.matmul(
                            ps[:, :], wr[:, off, coh * 128:(coh + 1) * 128],
                            f_pad[:, t * 8 + dy:t * 8 + dy + 8, dx:dx + W],
                            start=(off == 0), stop=(off == 8))
                    sc = bn[:, 6 + 2 * coh:7 + 2 * coh]
                    bi = bn[:, 7 + 2 * coh:8 + 2 * coh]
                    rr = sm.tile([128, 512], F32, name=f"rr_{coh}_{t}", tag="rr")
                    nc.scalar.activation(rr[:, :], ps[:, :], AF.Relu, bias=bi, scale=sc)
                    nc.vector.tensor_tensor(
                        out_sb[:, coh, t * 512:(t + 1) * 512], rr[:, :],
                        res[:, coh, t * 512:(t + 1) * 512], op=ALU.add)
            nc.sync.dma_start(out=out_d[:, :, :], in_=out_sb[:, :, :])

            if dump:
                nc.sync.dma_start(out=dq_d[:, :], in_=q_aug[:, :].bitcast(F32))
                nc.sync.dma_start(out=dk_d[:, :], in_=k_aug[:, :].bitcast(F32))
                nc.sync.dma_start(out=dv_d[:, :], in_=v_bf[:, :])
                nc.sync.dma_start(out=dm_d[:, :], in_=mcol[:, :])
                nc.sync.dma_start(out=de_d[:, :], in_=de_sb[:, :])
                df_sb = per.tile([65, NLOC], F32)
                for c in range(5):
                    nc.vector.tensor_copy(df_sb[:, cstart[c]:cstart[c + 1]],
                                          fbanks[c][:, :])
                nc.sync.dma_start(out=df_d[:, :], in_=df_sb[:, :])
                nc.sync.dma_start(out=dfp_d[:, :],
                                  in_=f_pad[:, :, :].bitcast(F32))
                nc.sync.dma_start(out=dvt_d[:, :], in_=vT[:, :, 0:65])

    nc.compile()
    return nc


_PROGRAM = None


def _get_program():
    global _PROGRAM
    if _PROGRAM is None:
        _PROGRAM = _build_program()
    return _PROGRAM


def _prep_core_inputs(inputs, b, h):
    f1 = np.asarray(inputs["feature1"][b], np.float32)     # [256, 64, 64]
    f2 = np.asarray(inputs["feature2"][b], np.float32)
    R0 = 32 * h

    f1p = np.pad(f1, ((0, 0), (1, 1), (1, 1)))             # [256, 66, 66]
    x1 = rne12(f1p).reshape(2, 128, 66, 66).transpose(1, 0, 2, 3)

    # x2 rows: global R0-2 .. R0+33 (36 rows), zeros outside [0, 64)
    f2p = np.zeros((256, 36, 66), np.float32)
    lo, hi = max(0, R0 - 2), min(64, R0 + 34)
    f2p[:, lo - (R0 - 2):hi - (R0 - 2), 1:65] = f2[:, lo:hi, :]
    x2 = rne12(f2p).reshape(2, 128, 36, 66).transpose(1, 0, 2, 3)

    # conv weights -> lhsT [ci, co] per (offset, ci_half)
    def lhsT(nm):
        w = np.asarray(inputs[nm], np.float32)             # [64, 256, 3, 3]
        wt = w.transpose(2, 3, 1, 0).reshape(9, 2, 128, 64)
        return wt.transpose(2, 0, 1, 3)                    # [128, 9, 2, 64]
    wq = rne12(lhsT("qw"))
    wkv = rne12(np.concatenate([lhsT("kw"), lhsT("vw")], axis=3))  # [128,9,2,128]
    wrr = np.asarray(inputs["rw"], np.float32)             # [256, 64, 3, 3]
    wr = rne12(np.ascontiguousarray(wrr.transpose(2, 3, 1, 0).reshape(9, 64, 256)
                                    .transpose(1, 0, 2)))

    # bn cols: 0/1 = q scale/bias (parts 0..63); 2/3 = k (parts 0..63) and
    # v (parts 64..127) scale/bias; 6..9 = r conv halves.
    bnv = np.zeros((128, 10), np.float32)
    for p, rows, cols in [("q", slice(0, 64), (0, 1)),
                          ("k", slice(0, 64), (2, 3)),
                          ("v", slice(64, 128), (2, 3))]:
        inv = inputs[p + "g"] / np.sqrt(inputs[p + "v"] + EPS)
        bias = inputs[p + "b"] * inv + inputs[p + "be"] - inputs[p + "m"] * inv
        bnv[rows, cols[0]] = inv
        bnv[rows, cols[1]] = bias
    rinv = inputs["rg"] / np.sqrt(inputs["rv"] + EPS)
    rbias = inputs["rb"] * rinv + inputs["rbe"] - inputs["rm"] * rinv
    bnv[:, 6], bnv[:, 7] = rinv[0:128], rbias[0:128]
    bnv[:, 8], bnv[:, 9] = rinv[128:256], rbias[128:256]

    resv = np.ascontiguousarray(
        f1[:, R0:R0 + 32, :].reshape(2, 128, 2048))
    resv = np.ascontiguousarray(resv.transpose(1, 0, 2))

    mask = np.ones((1, NLOC), np.float32)
    if h == 0:
        mask[0, 0:W] = 0.0          # attn row 0 = global row -1
    else:
        mask[0, (NROWS - 1) * W:] = 0.0   # attn row 33 = global row 64

    return {"x1": np.ascontiguousarray(x1), "x2": np.ascontiguousarray(x2),
            "wq": np.ascontiguousarray(wq), "wkv": np.ascontiguousarray(wkv),
            "wr": wr, "bn": bnv, "res": resv, "mask": mask}


def kernel(**inputs):
    nc = _get_program()
    in_maps = [_prep_core_inputs(inputs, core // 2, core % 2) for core in range(8)]
    results = run_bass_kernel_spmd(nc, in_maps, list(range(8))).results
    out = np.empty((4, 256, 64, 64), np.float32)
    for core in range(8):
        b, hh = core // 2, core % 2
        o = results[core]["out"]                  # [128, 2, 2048]
        o = o.transpose(1, 0, 2).reshape(256, 32, 64)
        out[b, :, 32 * hh:32 * hh + 32, :] = o
    return out


if __name__ == "__main__":
    rng = np.random.default_rng(0)
    ins = {}
    ins["feature1"] = rng.normal(size=(4, 256, 64, 64)).astype(np.float32)
    ins["feature2"] = rng.normal(size=(4, 256, 64, 64)).astype(np.float32)
    for p, cin, cout in [("q", 256, 64), ("k", 256, 64), ("v", 256, 64),
                         ("r", 64, 256)]:
        ins[p + "w"] = (rng.normal(size=(cout, cin, 3, 3)) * 0.05).astype(np.float32)
        ins[p + "b"] = np.zeros(cout, np.float32)
        ins[p + "g"] = np.ones(cout, np.float32)
        ins[p + "be"] = np.zeros(cout, np.float32)
        ins[p + "m"] = np.zeros(cout, np.float32)
        ins[p + "v"] = np.ones(cout, np.float32)
    out = kernel(**ins)
    print("ran", out.shape, out.dtype, np.abs(out).mean())



# revision 3
# speedup vs baseline: 4.0405x; 4.0405x over previous
"""Trainium2 Bass kernel for CrossModalityPositionAttention.

Model (per batch element b of 4):
  q = ConvBNReLU(feature2[b]; qw)   [64, 64, 64]
  k = ConvBNReLU(feature1[b]; kw)
  v = ConvBNReLU(feature1[b]; vw)
  attn = softmax(q^T k over channels), f = v @ attn^T
  out = feature1[b] + ConvBNReLU(f; rw)   [256, 64, 64]

Sharding: 8 cores = 4 batches x 2 spatial halves. Each core computes 34
attention rows (its 32 output rows + 1-row halo on each side; out-of-image
halo rows are masked to zero) against all 4096 key positions, then the
final conv for its own 32 rows. The residual add runs on the host.

The per-call wall clock is dominated by the host<->device link, so the
runtime keeps a single jitted shard_map executable alive across calls,
caches all weight-derived device buffers (re-uploaded only when the weight
bytes change), ships features in fp16 (10-bit mantissa, ~5e-4 rel — the
near-one-hot softmax needs q/k logits accurate to ~0.1 absolute, which
bf16 would miss), and reads back the pre-residual f in fp16.

Numerics: convs and score matmuls run in fp16 inputs with fp32 PSUM
accumulation; attention probabilities and the attn@v matmul run in bf16.
Softmax uses a shifted-exp with a per-row shift alpha[n] = max(S[n, ::8])
+ 45 (sampled row max + margin), with the shift injected as an extra
contraction channel (k row of ones, q row of -alpha) so exp(S - alpha)
reads straight out of PSUM. A row of ones appended to v^T makes the same
matmul accumulate sum(exp) for the final normalization. The alpha shift
cancels exactly in the normalization, so its fp16 rounding is harmless.
"""

import sys

sys.path.insert(0, "/opt/trn_rl_repo")

import hashlib

import numpy as np

import concourse.bacc as bacc
import concourse.mybir as mybir
from concourse import tile

F16 = mybir.dt.float16
F32 = mybir.dt.float32
BF16 = mybir.dt.bfloat16
AF = mybir.ActivationFunctionType
ALU = mybir.AluOpType

EPS = 1e-5
ALPHA_MARGIN = 45.0
H = W = 64
NROWS = 34                # attention rows per core (32 + 2 halo)
NLOC = NROWS * W          # 2176
NK = H * W                # 4096 key positions
CHUNK_ROWS = [7, 7, 7, 7, 6]        # attention n-chunks (x64 cols)
MTILES = NK // 128        # 32
NCORES = 8

WEIGHT_KEYS = [p + s for p in "qkvr" for s in ("w", "b", "g", "be", "m", "v")]


def _build_program():
    nc = bacc.Bacc("TRN2", target_bir_lowering=False, debug=False)

    x1_d = nc.dram_tensor("x1", [128, 2, 66, 66], F16, kind="ExternalInput")
    x2_d = nc.dram_tensor("x2", [128, 2, 36, 66], F16, kind="ExternalInput")
    wq_d = nc.dram_tensor("wq", [128, 9, 2, 64], F16, kind="ExternalInput")
    wkv_d = nc.dram_tensor("wkv", [128, 9, 2, 128], F16, kind="ExternalInput")
    wr_d = nc.dram_tensor("wr", [64, 9, 256], F16, kind="ExternalInput")
    bn_d = nc.dram_tensor("bn", [128, 10], F32, kind="ExternalInput")
    mask_d = nc.dram_tensor("mask", [1, NLOC], F32, kind="ExternalInput")
    out_d = nc.dram_tensor("out", [128, 2, 2048], F16, kind="ExternalOutput")

    with tile.TileContext(nc) as tc:
        with tc.tile_pool(name="per", bufs=1) as per, \
             tc.tile_pool(name="eb", bufs=4) as eb, \
             tc.tile_pool(name="sm", bufs=2) as sm, \
             tc.tile_pool(name="tp", bufs=3, space="PSUM") as tp, \
             tc.tile_pool(name="fp", bufs=1, space="PSUM") as fp:

            # ---- persistent SBUF tiles ----
            x1 = per.tile([128, 2, 66, 66], F16)
            x2 = per.tile([128, 2, 36, 66], F16)
            wq = per.tile([128, 9, 2, 64], F16)
            wkv = per.tile([128, 9, 2, 128], F16)
            wr = per.tile([64, 9, 256], F16)
            bn = per.tile([128, 10], F32)
            maskrow = per.tile([1, NLOC], F32)
            q_aug = per.tile([65, NLOC], F16)
            k_aug = per.tile([65, NK], F16)
            v_bf = per.tile([128, NK], BF16)   # v lives at partitions 64..127
            vT = per.tile([128, MTILES, 80], BF16)  # 80: 32B-aligned tile stride for DMA-transpose dests
            f_pad = per.tile([64, NROWS, 66], F16)
            mcol = per.tile([128, 17], F32)
            nacol = per.tile([128, 17], F32)
            na_f32 = per.tile([1, NLOC], F32)
            out_sb = per.tile([128, 2, 2048], F16)

            nc.sync.dma_start(out=wkv[:, :, :, :], in_=wkv_d[:, :, :, :])
            nc.sync.dma_start(out=wq[:, :, :, :], in_=wq_d[:, :, :, :])
            nc.sync.dma_start(out=bn[:, :], in_=bn_d[:, :])
            # x1/x2 in row slabs so the first conv tiles start early
            for half in range(2):
                nc.sync.dma_start(out=x1[:, half, 0:19, :], in_=x1_d[:, half, 0:19, :])
                nc.sync.dma_start(out=x1[:, half, 19:35, :],
                                  in_=x1_d[:, half, 19:35, :])
                nc.sync.dma_start(out=x1[:, half, 35:50, :],
                                  in_=x1_d[:, half, 35:50, :])
                nc.sync.dma_start(out=x1[:, half, 50:66, :],
                                  in_=x1_d[:, half, 50:66, :])
            for half in range(2):
                nc.sync.dma_start(out=x2[:, half, 0:18, :], in_=x2_d[:, half, 0:18, :])
                nc.sync.dma_start(out=x2[:, half, 18:36, :],
                                  in_=x2_d[:, half, 18:36, :])
            nc.sync.dma_start(out=maskrow[:, :], in_=mask_d[:, :])
            nc.sync.dma_start(out=wr[:, :, :], in_=wr_d[:, :, :])

            nc.vector.memset(k_aug[64:65, :], 1.0)
            nc.vector.memset(vT[:, :, 64:65], 1.0)
            nc.vector.memset(f_pad[:, :, :], 0.0)

            # ---- fused k+v conv (M=128: co 0..63 = k, 64..127 = v) ----
            for t in range(8):
                r0 = t * 8
                ps = tp.tile([128, 512], F32, name=f"kv_{t}", tag="tpsum")
                for half in range(2):
                    for off in range(9):
                        dy, dx = off // 3, off % 3
                        nc.tensor.matmul(
                            ps[:, :], wkv[:, off, half, :],
                            x1[:, half, r0 + dy:r0 + dy + 8, dx:dx + W],
                            start=(half == 0 and off == 0),
                            stop=(half == 1 and off == 8))
                nc.scalar.activation(k_aug[0:64, r0 * W:(r0 + 8) * W], ps[0:64, :],
                                     AF.Relu, bias=bn[0:64, 3:4], scale=bn[0:64, 2:3])
                nc.scalar.activation(v_bf[64:128, r0 * W:(r0 + 8) * W], ps[64:128, :],
                                     AF.Relu, bias=bn[64:128, 3:4],
                                     scale=bn[64:128, 2:3])
                # v^T for this 512-col span (4 m-tiles) via DMA transpose
                for mt in range(t * 4, t * 4 + 4):
                    nc.sync.dma_start(out=vT[:, mt, 0:64],
                                      in_=v_bf[64:128, mt * 128:(mt + 1) * 128],
                                      transpose=True)

            # ---- q conv (M=64) ----
            r0 = 0
            sub_done = []
            for t, rows in enumerate(CHUNK_ROWS):
                na = rows * W
                ps = tp.tile([128, 512], F32, name=f"qc_{t}", tag="tpsum")
                for half in range(2):
                    for off in range(9):
                        dy, dx = off // 3, off % 3
                        nc.tensor.matmul(
                            ps[0:64, 0:na], wq[:, off, half, :],
                            x2[:, half, r0 + dy:r0 + dy + rows, dx:dx + W],
                            start=(half == 0 and off == 0),
                            stop=(half == 1 and off == 8))
                nc.scalar.activation(q_aug[0:64, r0 * W:r0 * W + na], ps[0:64, 0:na],
                                     AF.Relu, bias=bn[0:64, 1:2], scale=bn[0:64, 0:1])
                r0 += rows
                # sampled row-max S_sub tiles whose q columns are now ready
                while len(sub_done) < 17 and (len(sub_done) + 1) * 128 <= r0 * W:
                    st_ = len(sub_done)
                    sps = tp.tile([128, 512], F32, name=f"sub_{st_}", tag="tpsum")
                    nc.tensor.matmul(sps[:, :],
                                     q_aug[0:64, st_ * 128:(st_ + 1) * 128],
                                     k_aug[0:64, ::8], start=True, stop=True)
                    nc.vector.tensor_reduce(mcol[:, st_:st_ + 1], sps[:, :],
                                            axis=mybir.AxisListType.X, op=ALU.max)
                    sub_done.append(st_)

            # -alpha = -(submax + MARGIN)
            nc.vector.tensor_scalar(nacol[:, :], mcol[:, :], -1.0, -ALPHA_MARGIN,
                                    ALU.mult, ALU.add)
            for t in range(17):
                nc.sync.dma_start(out=na_f32[:, t * 128:(t + 1) * 128],
                                  in_=nacol[:, t:t + 1])
            nc.vector.tensor_copy(q_aug[64:65, :], na_f32[:, :])

            # ---- attention: S^T -> exp -> attn @ v (+ sumexp row) ----
            cstart = [0]
            for r in CHUNK_ROWS:
                cstart.append(cstart[-1] + r * W)
            fbanks = [fp.tile([65, CHUNK_ROWS[c] * W], F32, name=f"fb{c}")
                      for c in range(5)]
            for m in range(MTILES):
                for c in range(5):
                    n0, n1 = cstart[c], cstart[c + 1]
                    st = tp.tile([128, 512], F32, name=f"st_{m}_{c}", tag="tpsum")
                    nc.tensor.matmul(st[:, 0:n1 - n0], k_aug[:, m * 128:(m + 1) * 128],
                                     q_aug[:, n0:n1], start=True, stop=True)
                    e = eb.tile([128, 512], BF16, name=f"e_{m}_{c}", tag="ebuf")
                    nc.scalar.activation(e[:, 0:n1 - n0], st[:, 0:n1 - n0], AF.Exp)
                    nc.tensor.matmul(fbanks[c][:, :], vT[:, m, 0:65], e[:, 0:n1 - n0],
                                     start=(m == 0), stop=(m == MTILES - 1))

            # ---- normalize f and store into padded layout ----
            for c in range(5):
                n0, n1 = cstart[c], cstart[c + 1]
                rcp = sm.tile([1, 512], F32, name=f"rcp{c}", tag="rcp")
                nc.vector.reciprocal(rcp[:, 0:n1 - n0], fbanks[c][64:65, :])
                rcpm = sm.tile([1, 512], F32, name=f"rcpm{c}", tag="rcpm")
                nc.vector.tensor_tensor(rcpm[:, 0:n1 - n0], rcp[:, 0:n1 - n0],
                                        maskrow[:, n0:n1], op=ALU.mult)
                rb = sm.tile([64, 512], F32, name=f"rb{c}", tag="rb")
                nc.gpsimd.partition_broadcast(rb[:, 0:n1 - n0], rcpm[:, 0:n1 - n0])
                row0 = n0 // W
                nc.vector.tensor_tensor(
                    f_pad[:, row0:row0 + CHUNK_ROWS[c], 1:65],
                    fbanks[c][0:64, :], rb[:, 0:n1 - n0], op=ALU.mult)

            # ---- final conv(64->256) + BN + ReLU (residual added on host) ----
            for coh in range(2):
                for t in range(4):
                    ps = tp.tile([128, 512], F32, name=f"rps_{coh}_{t}", tag="tpsum")
                    for off in range(9):
                        dy, dx = off // 3, off % 3
                        nc.tensor.matmul(
                            ps[:, :], wr[:, off, coh * 128:(coh + 1) * 128],
                            f_pad[:, t * 8 + dy:t * 8 + dy + 8, dx:dx + W],
                            start=(off == 0), stop=(off == 8))
                    sc = bn[:, 6 + 2 * coh:7 + 2 * coh]
                    bi = bn[:, 7 + 2 * coh:8 + 2 * coh]
                    nc.scalar.activation(out_sb[:, coh, t * 512:(t + 1) * 512],
                                         ps[:, :], AF.Relu, bias=bi, scale=sc)
            nc.sync.dma_start(out=out_d[:, :, :], in_=out_sb[:, :, :])

    nc.compile()
    return nc


class _Runtime:
    def __init__(self):
        import jax
        from jax.sharding import Mesh, NamedSharding, PartitionSpec
        from jax.experimental.shard_map import shard_map
        from concourse.bass2jax import (_bass_exec_p, install_neuronx_cc_hook,
                                        partition_id_tensor)

        self.jax = jax
        install_neuronx_cc_hook()
        nc = _build_program()
        self.nc = nc

        partition_name = (nc.partition_id_tensor.name
                          if nc.partition_id_tensor else None)
        in_names, out_names, out_avals = [], [], []
        for alloc in nc.m.functions[0].allocations:
            if not isinstance(alloc, mybir.MemoryLocationSet):
                continue
            name = alloc.memorylocations[0].name
            if alloc.kind == "ExternalInput":
                if name != partition_name:
                    in_names.append(name)
            elif alloc.kind == "ExternalOutput":
                out_names.append(name)
                out_avals.append(jax.core.ShapedArray(
                    tuple(alloc.tensor_shape), mybir.dt.np(alloc.dtype)))
        self.in_names = in_names
        n_in = len(in_names) + len(out_names)
        all_in_names = in_names + out_names + (
            [partition_name] if partition_name else [])

        def _body(*args):
            operands = list(args)
            if partition_name is not None:
                operands.append(partition_id_tensor())
            outs = _bass_exec_p.bind(
                *operands, out_avals=tuple(out_avals),
                in_names=tuple(all_in_names), out_names=tuple(out_names),
                lowering_input_output_aliases=(), sim_require_finite=True,
                sim_require_nnan=True, nc=nc)
            return tuple(outs)

        devices = jax.devices()[:NCORES]
        mesh = Mesh(np.asarray(devices), ("core",))
        self.sharding = NamedSharding(mesh, PartitionSpec("core"))
        self.fn = jax.jit(shard_map(
            _body, mesh=mesh, in_specs=(PartitionSpec("core"),) * n_in,
            out_specs=(PartitionSpec("core"),) * len(out_names),
            check_rep=False))

        # The NEFF writes every element of `out`, so the output operand only
        # has to exist — a persistent non-donated dummy avoids shipping
        # fresh zero buffers on every call.
        self.dummy_out = jax.device_put(
            np.zeros((NCORES * 128, 2, 2048), np.float16), self.sharding)

        # persistent pinned feature staging buffers (borders stay zero)
        self.x1_host = np.zeros((NCORES * 128, 2, 66, 66), np.float16)
        self.x2_host = np.zeros((NCORES * 128, 2, 36, 66), np.float16)

        self.weight_digest = None
        self.weight_dev = None

    def upload_weights(self, inputs):
        h = hashlib.blake2b(digest_size=16)
        arrs = {k: np.asarray(inputs[k], np.float32) for k in WEIGHT_KEYS}
        for k in WEIGHT_KEYS:
            h.update(np.ascontiguousarray(arrs[k]).tobytes())
        digest = h.digest()
        if digest == self.weight_digest:
            return
        # conv weights -> lhsT [ci, co] per (offset, ci_half)
        def lhsT(nm):
            w = arrs[nm]                                    # [64, 256, 3, 3]
            wt = w.transpose(2, 3, 1, 0).reshape(9, 2, 128, 64)
            return wt.transpose(2, 0, 1, 3)                 # [128, 9, 2, 64]
        wq = lhsT("qw").astype(np.float16)
        wkv = np.concatenate([lhsT("kw"), lhsT("vw")], axis=3).astype(np.float16)
        wr = np.ascontiguousarray(
            arrs["rw"].transpose(2, 3, 1, 0).reshape(9, 64, 256)
            .transpose(1, 0, 2)).astype(np.float16)         # [64, 9, 256]

        # bn cols: 0/1 = q scale/bias (parts 0..63); 2/3 = k (parts 0..63)
        # and v (parts 64..127) scale/bias; 6..9 = r conv halves.
        bnv = np.zeros((128, 10), np.float32)
        for p, rows, cols in [("q", slice(0, 64), (0, 1)),
                              ("k", slice(0, 64), (2, 3)),
                              ("v", slice(64, 128), (2, 3))]:
            inv = arrs[p + "g"] / np.sqrt(arrs[p + "v"] + EPS)
            bias = arrs[p + "b"] * inv + arrs[p + "be"] - arrs[p + "m"] * inv
            bnv[rows, cols[0]] = inv
            bnv[rows, cols[1]] = bias
        rinv = arrs["rg"] / np.sqrt(arrs["rv"] + EPS)
        rbias = arrs["rb"] * rinv + arrs["rbe"] - arrs["rm"] * rinv
        bnv[:, 6], bnv[:, 7] = rinv[0:128], rbias[0:128]
        bnv[:, 8], bnv[:, 9] = rinv[128:256], rbias[128:256]

        # halo-row mask: per-core, rows outside the image are zeroed
        mask = np.ones((NCORES, 1, NLOC), np.float32)
        for core in range(NCORES):
            if core % 2 == 0:
                mask[core, 0, 0:W] = 0.0            # attn row 0 = global row -1
            else:
                mask[core, 0, (NROWS - 1) * W:] = 0.0   # row 33 = global row 64

        def rep(a):  # replicate a per-core array for all 8 cores
            return np.ascontiguousarray(
                np.broadcast_to(a[None], (NCORES,) + a.shape)
            ).reshape((NCORES * a.shape[0],) + a.shape[1:])

        dev = {}
        for name, arr in [("wq", rep(wq)), ("wkv", rep(wkv)), ("wr", rep(wr)),
                          ("bn", rep(bnv)),
                          ("mask", mask.reshape(NCORES * 1, NLOC))]:
            dev[name] = self.jax.device_put(arr, self.sharding)
        self.jax.block_until_ready(list(dev.values()))
        self.weight_dev = dev
        self.weight_digest = digest

    def stage_features(self, feature1, feature2):
        f1_16 = np.asarray(feature1).astype(np.float16)     # [4, 256, 64, 64]
        f2_16 = np.asarray(feature2).astype(np.float16)
        x1 = self.x1_host.reshape(NCORES, 128, 2, 66, 66)
        x2 = self.x2_host.reshape(NCORES, 128, 2, 36, 66)
        for b in range(4):
            src1 = f1_16[b].reshape(2, 128, 64, 64).transpose(1, 0, 2, 3)
            x1[2 * b, :, :, 1:65, 1:65] = src1
            x1[2 * b + 1, :, :, 1:65, 1:65] = src1
            src2 = f2_16[b].reshape(2, 128, 64, 64).transpose(1, 0, 2, 3)
            for hh in range(2):
                R0 = 32 * hh
                lo, hi = max(0, R0 - 2), min(64, R0 + 34)
                x2[2 * b + hh, :, :, lo - (R0 - 2):hi - (R0 - 2), 1:65] = \
                    src2[:, :, lo:hi, :]
        return (self.jax.device_put(self.x1_host, self.sharding),
                self.jax.device_put(self.x2_host, self.sharding))

    def __call__(self, inputs):
        self.upload_weights(inputs)
        x1_dev, x2_dev = self.stage_features(inputs["feature1"],
                                             inputs["feature2"])
        dev = {"x1": x1_dev, "x2": x2_dev, **self.weight_dev}
        outs = self.fn(*[dev[nm] for nm in self.in_names], self.dummy_out)
        out_np = np.asarray(outs[0])                 # [1024, 2, 2048] fp16
        # [b, hh, p, half, r, x] -> [b, half, p, hh, r, x]; ch = 128*half + p
        f = np.ascontiguousarray(
            out_np.reshape(4, 2, 128, 2, 32, 64).transpose(0, 3, 2, 1, 4, 5)
        ).reshape(4, 256, 64, 64)
        return (np.asarray(inputs["feature1"], np.float32) + f).astype(
            np.float32, copy=False)


_RT = None


def kernel(**inputs):
    global _RT
    if _RT is None:
        _RT = _Runtime()
    return _RT(inputs)


if __name__ == "__main__":
    rng = np.random.default_rng(0)
    ins = {}
    ins["feature1"] = rng.normal(size=(4, 256, 64, 64)).astype(np.float32)
    ins["feature2"] = rng.normal(size=(4, 256, 64, 64)).astype(np.float32)
    for p, cin, cout in [("q", 256, 64), ("k", 256, 64), ("v", 256, 64),
                         ("r", 64, 256)]:
        ins[p + "w"] = (rng.normal(size=(cout, cin, 3, 3)) * 0.05).astype(np.float32)
        ins[p + "b"] = np.zeros(cout, np.float32)
        ins[p + "g"] = np.ones(cout, np.float32)
        ins[p + "be"] = np.zeros(cout, np.float32)
        ins[p + "m"] = np.zeros(cout, np.float32)
        ins[p + "v"] = np.ones(cout, np.float32)
    out = kernel(**ins)
    print("ran", out.shape, out.dtype, np.abs(out).mean())


# revision 4
# speedup vs baseline: 6.5281x; 1.6156x over previous
"""Trainium2 Bass kernel for CrossModalityPositionAttention.

Model (per batch element b of 4):
  q = ConvBNReLU(feature2[b]; qw)   [64, 64, 64]
  k = ConvBNReLU(feature1[b]; kw)
  v = ConvBNReLU(feature1[b]; vw)
  attn = softmax(q^T k over channels), f = v @ attn^T
  out = feature1[b] + ConvBNReLU(f; rw)   [256, 64, 64]

Sharding: 4 cores, one full batch element per core (cores 4..7 idle). The
per-call wall clock is dominated by the host<->device axon link (~10ms/MB
up, ~25ms/MB down, ~75ms fixed per transfer/launch), not by device
compute (~0.3ms of PE work per core), so the layout minimizes link bytes:

  - one fp16 upload holding exactly one copy of each feature map (16.8MB
    total, the information-theoretic floor at fp16), in half-major
    [2,128,64,64] dram layout so host staging is a contiguous
    memcpy-with-cast (no transpose, no padding bytes — the device pads
    via memset + interior DMA);
  - all weight-derived buffers live on device, re-uploaded only when the
    weight bytes change (blake2b check);
  - the output is the pre-residual f in fp16 ([2,128,4096] half-major,
    8.4MB total); the exact-fp32 residual add happens on the host;
  - a single jitted shard_map executable is built once and reused.

Numerics: fp16 features/weights (10-bit mantissa, ~5e-4 rel — the
near-one-hot softmax needs q/k logits accurate to ~0.1 absolute, which
bf16's 8-bit mantissa would miss), fp32 PSUM accumulation everywhere;
attention probabilities and the attn@v matmul run in bf16 (needed for
exp range). Softmax uses a shifted exp with per-row shift alpha[n] =
max(S[n, ::8]) + 45 (sampled row max + margin), injected as an extra
contraction channel (k row of ones, q row of -alpha) so exp(S - alpha)
reads straight out of PSUM; a row of ones appended to v^T makes the same
matmul accumulate sum(exp). The alpha shift cancels exactly in the
normalization, so its fp16 rounding is harmless.
"""

import sys

sys.path.insert(0, "/opt/trn_rl_repo")

import hashlib

import numpy as np

import concourse.bacc as bacc
import concourse.mybir as mybir
from concourse import tile

F16 = mybir.dt.float16
F32 = mybir.dt.float32
BF16 = mybir.dt.bfloat16
AF = mybir.ActivationFunctionType
ALU = mybir.AluOpType

EPS = 1e-5
ALPHA_MARGIN = 45.0
H = W = 64
N = H * W                 # 4096 positions (attention rows and keys)
MTILES = N // 128         # 32
NCORES = 4

WEIGHT_KEYS = [p + s for p in "qkvr" for s in ("w", "b", "g", "be", "m", "v")]


def _build_program():
    nc = bacc.Bacc("TRN2", target_bir_lowering=False, debug=False)

    # xx: [f1 half0, f1 half1, f2 half0, f2 half1], each [128, 64, 64]
    xx_d = nc.dram_tensor("xx", [4, 128, 64, 64], F16, kind="ExternalInput")
    wq_d = nc.dram_tensor("wq", [128, 9, 2, 64], F16, kind="ExternalInput")
    wkv_d = nc.dram_tensor("wkv", [128, 9, 2, 128], F16, kind="ExternalInput")
    wr_d = nc.dram_tensor("wr", [64, 9, 256], F16, kind="ExternalInput")
    bn_d = nc.dram_tensor("bn", [128, 10], F32, kind="ExternalInput")
    out_d = nc.dram_tensor("out", [2, 128, 4096], F16, kind="ExternalOutput")

    with tile.TileContext(nc) as tc:
        with tc.tile_pool(name="per", bufs=1) as per, \
             tc.tile_pool(name="eb", bufs=4) as eb, \
             tc.tile_pool(name="sm", bufs=2) as sm, \
             tc.tile_pool(name="tp", bufs=3, space="PSUM") as tp, \
             tc.tile_pool(name="fp", bufs=4, space="PSUM") as fp:

            # ---- persistent SBUF tiles ----
            x1 = per.tile([128, 2, 66, 66], F16)
            x2 = per.tile([128, 2, 66, 66], F16)
            wq = per.tile([128, 9, 2, 64], F16)
            wkv = per.tile([128, 9, 2, 128], F16)
            wr = per.tile([64, 9, 256], F16)
            bn = per.tile([128, 10], F32)
            q_aug = per.tile([65, N], F16)
            k_aug = per.tile([65, N], F16)
            v_bf = per.tile([128, N], BF16)    # v lives at partitions 64..127
            vT = per.tile([128, MTILES, 80], BF16)  # 80: 32B-aligned stride for DMA-transpose dests
            f_pad = per.tile([64, 66, 66], F16)
            mcol = per.tile([128, 32], F32)
            nacol = per.tile([128, 32], F32)
            na_f32 = per.tile([1, N], F32)
            out_sb = per.tile([128, 2, 4096], F16)

            nc.sync.dma_start(out=wkv[:, :, :, :], in_=wkv_d[:, :, :, :])
            nc.sync.dma_start(out=wq[:, :, :, :], in_=wq_d[:, :, :, :])
            nc.sync.dma_start(out=bn[:, :], in_=bn_d[:, :])

            # zero the padded borders, then land the raw features in the
            # interior; row slabs so the first conv tiles can start early
            nc.vector.memset(x1[:, :, :, :], 0.0)
            nc.vector.memset(x2[:, :, :, :], 0.0)
            for half in range(2):
                for r0, r1 in [(0, 18), (18, 34), (34, 49), (49, 64)]:
                    nc.sync.dma_start(out=x1[:, half, 1 + r0:1 + r1, 1:65],
                                      in_=xx_d[half, :, r0:r1, :])
            for half in range(2):
                for r0, r1 in [(0, 32), (32, 64)]:
                    nc.sync.dma_start(out=x2[:, half, 1 + r0:1 + r1, 1:65],
                                      in_=xx_d[2 + half, :, r0:r1, :])
            nc.sync.dma_start(out=wr[:, :, :], in_=wr_d[:, :, :])

            nc.vector.memset(k_aug[64:65, :], 1.0)
            nc.vector.memset(vT[:, :, 64:65], 1.0)
            nc.vector.memset(f_pad[:, :, :], 0.0)

            # ---- fused k+v conv (M=128: co 0..63 = k, 64..127 = v) ----
            for t in range(8):
                r0 = t * 8
                ps = tp.tile([128, 512], F32, name=f"kv_{t}", tag="tpsum")
                for half in range(2):
                    for off in range(9):
                        dy, dx = off // 3, off % 3
                        nc.tensor.matmul(
                            ps[:, :], wkv[:, off, half, :],
                            x1[:, half, r0 + dy:r0 + dy + 8, dx:dx + W],
                            start=(half == 0 and off == 0),
                            stop=(half == 1 and off == 8))
                nc.scalar.activation(k_aug[0:64, r0 * W:(r0 + 8) * W], ps[0:64, :],
                                     AF.Relu, bias=bn[0:64, 3:4], scale=bn[0:64, 2:3])
                nc.scalar.activation(v_bf[64:128, r0 * W:(r0 + 8) * W], ps[64:128, :],
                                     AF.Relu, bias=bn[64:128, 3:4],
                                     scale=bn[64:128, 2:3])
                # v^T for this 512-col span (4 m-tiles) via DMA transpose
                for mt in range(t * 4, t * 4 + 4):
                    nc.sync.dma_start(out=vT[:, mt, 0:64],
                                      in_=v_bf[64:128, mt * 128:(mt + 1) * 128],
                                      transpose=True)

            # ---- q conv (M=64), interleaved with sampled row-max tiles ----
            for t in range(8):
                r0 = t * 8
                ps = tp.tile([128, 512], F32, name=f"qc_{t}", tag="tpsum")
                for half in range(2):
                    for off in range(9):
                        dy, dx = off // 3, off % 3
                        nc.tensor.matmul(
                            ps[0:64, :], wq[:, off, half, :],
                            x2[:, half, r0 + dy:r0 + dy + 8, dx:dx + W],
                            start=(half == 0 and off == 0),
                            stop=(half == 1 and off == 8))
                nc.scalar.activation(q_aug[0:64, r0 * W:(r0 + 8) * W], ps[0:64, :],
                                     AF.Relu, bias=bn[0:64, 1:2], scale=bn[0:64, 0:1])
                # sampled row-max S_sub for the 4 fresh 128-col spans of q
                for st_ in range(t * 4, t * 4 + 4):
                    sps = tp.tile([128, 512], F32, name=f"sub_{st_}", tag="tpsum")
                    nc.tensor.matmul(sps[:, :],
                                     q_aug[0:64, st_ * 128:(st_ + 1) * 128],
                                     k_aug[0:64, ::8], start=True, stop=True)
                    nc.vector.tensor_reduce(mcol[:, st_:st_ + 1], sps[:, :],
                                            axis=mybir.AxisListType.X, op=ALU.max)

            # -alpha = -(submax + MARGIN), spread to a [1, N] row
            nc.vector.tensor_scalar(nacol[:, :], mcol[:, :], -1.0, -ALPHA_MARGIN,
                                    ALU.mult, ALU.add)
            for t in range(32):
                nc.sync.dma_start(out=na_f32[:, t * 128:(t + 1) * 128],
                                  in_=nacol[:, t:t + 1])
            nc.vector.tensor_copy(q_aug[64:65, :], na_f32[:, :])

            # ---- attention: S^T -> exp -> attn @ v (+ sumexp row) ----
            # two row-groups of 2048, each split into 4 chunks of 512 cols;
            # 4 PSUM f-banks rotate between the groups
            for g in range(2):
                fbanks = [fp.tile([65, 512], F32, name=f"fb_{g}_{c}", tag="fbank")
                          for c in range(4)]
                for m in range(MTILES):
                    for c in range(4):
                        n0 = g * 2048 + c * 512
                        st = tp.tile([128, 512], F32, name=f"st_{g}_{m}_{c}",
                                     tag="tpsum")
                        nc.tensor.matmul(st[:, :], k_aug[:, m * 128:(m + 1) * 128],
                                         q_aug[:, n0:n0 + 512],
                                         start=True, stop=True)
                        e = eb.tile([128, 512], BF16, name=f"e_{g}_{m}_{c}",
                                    tag="ebuf")
                        nc.scalar.activation(e[:, :], st[:, :], AF.Exp)
                        nc.tensor.matmul(fbanks[c][:, :], vT[:, m, 0:65], e[:, :],
                                         start=(m == 0), stop=(m == MTILES - 1))

                # normalize f and store into the padded conv-input layout
                for c in range(4):
                    n0 = g * 2048 + c * 512
                    row0 = n0 // W
                    rcp = sm.tile([1, 512], F32, name=f"rcp{g}{c}", tag="rcp")
                    nc.vector.reciprocal(rcp[:, :], fbanks[c][64:65, :])
                    rb = sm.tile([64, 512], F32, name=f"rb{g}{c}", tag="rb")
                    nc.gpsimd.partition_broadcast(rb[:, :], rcp[:, :])
                    nc.vector.tensor_tensor(
                        f_pad[:, 1 + row0:1 + row0 + 8, 1:65],
                        fbanks[c][0:64, :], rb[:, :], op=ALU.mult)

            # ---- final conv(64->256) + BN + ReLU (residual added on host) ----
            for coh in range(2):
                for t in range(8):
                    ps = tp.tile([128, 512], F32, name=f"rps_{coh}_{t}", tag="tpsum")
                    for off in range(9):
                        dy, dx = off // 3, off % 3
                        nc.tensor.matmul(
                            ps[:, :], wr[:, off, coh * 128:(coh + 1) * 128],
                            f_pad[:, t * 8 + dy:t * 8 + dy + 8, dx:dx + W],
                            start=(off == 0), stop=(off == 8))
                    sc = bn[:, 6 + 2 * coh:7 + 2 * coh]
                    bi = bn[:, 7 + 2 * coh:8 + 2 * coh]
                    nc.scalar.activation(out_sb[:, coh, t * 512:(t + 1) * 512],
                                         ps[:, :], AF.Relu, bias=bi, scale=sc)
            for half in range(2):
                nc.sync.dma_start(out=out_d[half, :, :], in_=out_sb[:, half, :])

    nc.compile()
    return nc


class _Runtime:
    def __init__(self):
        import jax
        from jax.sharding import Mesh, NamedSharding, PartitionSpec
        from jax.experimental.shard_map import shard_map
        from concourse.bass2jax import (_bass_exec_p, install_neuronx_cc_hook,
                                        partition_id_tensor)

        self.jax = jax
        install_neuronx_cc_hook()
        nc = _build_program()
        self.nc = nc

        partition_name = (nc.partition_id_tensor.name
                          if nc.partition_id_tensor else None)
        in_names, out_names, out_avals = [], [], []
        for alloc in nc.m.functions[0].allocations:
            if not isinstance(alloc, mybir.MemoryLocationSet):
                continue
            name = alloc.memorylocations[0].name
            if alloc.kind == "ExternalInput":
                if name != partition_name:
                    in_names.append(name)
            elif alloc.kind == "ExternalOutput":
                out_names.append(name)
                out_avals.append(jax.core.ShapedArray(
                    tuple(alloc.tensor_shape), mybir.dt.np(alloc.dtype)))
        self.in_names = in_names
        n_in = len(in_names) + len(out_names)
        all_in_names = in_names + out_names + (
            [partition_name] if partition_name else [])

        def _body(*args):
            operands = list(args)
            if partition_name is not None:
                operands.append(partition_id_tensor())
            outs = _bass_exec_p.bind(
                *operands, out_avals=tuple(out_avals),
                in_names=tuple(all_in_names), out_names=tuple(out_names),
                lowering_input_output_aliases=(), sim_require_finite=True,
                sim_require_nnan=True, nc=nc)
            return tuple(outs)

        devices = jax.devices()[:NCORES]
        mesh = Mesh(np.asarray(devices), ("core",))
        self.sharding = NamedSharding(mesh, PartitionSpec("core"))
        self.fn = jax.jit(shard_map(
            _body, mesh=mesh, in_specs=(PartitionSpec("core"),) * n_in,
            out_specs=(PartitionSpec("core"),) * len(out_names),
            check_rep=False))

        # The NEFF writes every element of `out`, so the output operand only
        # has to exist — a persistent non-donated dummy avoids shipping
        # fresh zero buffers on every call.
        self.dummy_out = jax.device_put(
            np.zeros((NCORES * 2, 128, 4096), np.float16), self.sharding)

        # persistent pinned feature staging buffer
        self.xx_host = np.empty((NCORES * 4, 128, 64, 64), np.float16)

        self.weight_digest = None
        self.weight_dev = None

    def upload_weights(self, inputs):
        h = hashlib.blake2b(digest_size=16)
        arrs = {k: np.asarray(inputs[k], np.float32) for k in WEIGHT_KEYS}
        for k in WEIGHT_KEYS:
            h.update(np.ascontiguousarray(arrs[k]).tobytes())
        digest = h.digest()
        if digest == self.weight_digest:
            return
        # conv weights -> lhsT [ci, co] per (offset, ci_half)
        def lhsT(nm):
            w = arrs[nm]                                    # [64, 256, 3, 3]
            wt = w.transpose(2, 3, 1, 0).reshape(9, 2, 128, 64)
            return wt.transpose(2, 0, 1, 3)                 # [128, 9, 2, 64]
        wq = lhsT("qw").astype(np.float16)
        wkv = np.concatenate([lhsT("kw"), lhsT("vw")], axis=3).astype(np.float16)
        wr = np.ascontiguousarray(
            arrs["rw"].transpose(2, 3, 1, 0).reshape(9, 64, 256)
            .transpose(1, 0, 2)).astype(np.float16)         # [64, 9, 256]

        # bn cols: 0/1 = q scale/bias (parts 0..63); 2/3 = k (parts 0..63)
        # and v (parts 64..127) scale/bias; 6..9 = r conv halves.
        bnv = np.zeros((128, 10), np.float32)
        for p, rows, cols in [("q", slice(0, 64), (0, 1)),
                              ("k", slice(0, 64), (2, 3)),
                              ("v", slice(64, 128), (2, 3))]:
            inv = arrs[p + "g"] / np.sqrt(arrs[p + "v"] + EPS)
            bias = arrs[p + "b"] * inv + arrs[p + "be"] - arrs[p + "m"] * inv
            bnv[rows, cols[0]] = inv
            bnv[rows, cols[1]] = bias
        rinv = arrs["rg"] / np.sqrt(arrs["rv"] + EPS)
        rbias = arrs["rb"] * rinv + arrs["rbe"] - arrs["rm"] * rinv
        bnv[:, 6], bnv[:, 7] = rinv[0:128], rbias[0:128]
        bnv[:, 8], bnv[:, 9] = rinv[128:256], rbias[128:256]

        def rep(a):  # replicate a per-core array for all cores
            return np.ascontiguousarray(
                np.broadcast_to(a[None], (NCORES,) + a.shape)
            ).reshape((NCORES * a.shape[0],) + a.shape[1:])

        dev = {}
        for name, arr in [("wq", rep(wq)), ("wkv", rep(wkv)), ("wr", rep(wr)),
                          ("bn", rep(bnv))]:
            dev[name] = self.jax.device_put(arr, self.sharding)
        self.jax.block_until_ready(list(dev.values()))
        self.weight_dev = dev
        self.weight_digest = digest

    def __call__(self, inputs):
        self.upload_weights(inputs)
        f1 = np.asarray(inputs["feature1"], np.float32)
        f2 = np.asarray(inputs["feature2"], np.float32)
        # half-major staging: [b, {f1h0, f1h1, f2h0, f2h1}, 128, 64, 64];
        # contiguous assignment with inline fp32->fp16 cast
        xxv = self.xx_host.reshape(4, 4, 128, 64, 64)
        xxv[:, 0:2] = f1.reshape(4, 2, 128, 64, 64)
        xxv[:, 2:4] = f2.reshape(4, 2, 128, 64, 64)
        xx_dev = self.jax.device_put(self.xx_host, self.sharding)
        dev = {"xx": xx_dev, **self.weight_dev}
        outs = self.fn(*[dev[nm] for nm in self.in_names], self.dummy_out)
        out_np = np.asarray(outs[0])                 # [8, 128, 4096] fp16
        f = out_np.reshape(4, 2, 128, 64, 64)
        result = np.empty((4, 256, 64, 64), np.float32)
        np.add(f1.reshape(4, 2, 128, 64, 64), f,
               out=result.reshape(4, 2, 128, 64, 64))
        return result


_RT = None


def kernel(**inputs):
    global _RT
    if _RT is None:
        _RT = _Runtime()
    return _RT(inputs)


if __name__ == "__main__":
    rng = np.random.default_rng(0)
    ins = {}
    ins["feature1"] = rng.normal(size=(4, 256, 64, 64)).astype(np.float32)
    ins["feature2"] = rng.normal(size=(4, 256, 64, 64)).astype(np.float32)
    for p, cin, cout in [("q", 256, 64), ("k", 256, 64), ("v", 256, 64),
                         ("r", 64, 256)]:
        ins[p + "w"] = (rng.normal(size=(cout, cin, 3, 3)) * 0.05).astype(np.float32)
        ins[p + "b"] = np.zeros(cout, np.float32)
        ins[p + "g"] = np.ones(cout, np.float32)
        ins[p + "be"] = np.zeros(cout, np.float32)
        ins[p + "m"] = np.zeros(cout, np.float32)
        ins[p + "v"] = np.ones(cout, np.float32)
    out = kernel(**ins)
    print("ran", out.shape, out.dtype, np.abs(out).mean())


# revision 6
# speedup vs baseline: 6.6393x; 1.0170x over previous
"""Trainium2 Bass kernel for CrossModalityPositionAttention.

Model (per batch element b of 4):
  q = ConvBNReLU(feature2[b]; qw)   [64, 64, 64]
  k = ConvBNReLU(feature1[b]; kw)
  v = ConvBNReLU(feature1[b]; vw)
  attn = softmax(q^T k over channels), f = v @ attn^T
  out = feature1[b] + ConvBNReLU(f; rw)   [256, 64, 64]

Sharding: 4 cores, one full batch element per core (cores 4..7 idle). The
per-call wall clock is dominated by the host<->device axon link (~10ms/MB
up, ~25ms/MB down, ~75ms fixed per transfer/launch), not by device
compute (~0.3ms of PE work per core), so the layout minimizes link bytes:

  - one fp16 upload holding exactly one copy of each feature map (16.8MB
    total, the information-theoretic floor at fp16), in half-major
    [2,128,64,64] dram layout so host staging is a contiguous
    memcpy-with-cast (no transpose, no padding bytes — the device pads
    via memset + interior DMA);
  - all weight-derived buffers live on device, re-uploaded only when the
    weight bytes change (blake2b check);
  - the output is the pre-residual f in fp16 ([2,128,4096] half-major,
    8.4MB total); the exact-fp32 residual add happens on the host;
  - a single jitted shard_map executable is built once and reused.

Numerics: fp16 features/weights (10-bit mantissa, ~5e-4 rel — the
near-one-hot softmax needs q/k logits accurate to ~0.1 absolute, which
bf16's 8-bit mantissa would miss), fp32 PSUM accumulation everywhere;
attention probabilities and the attn@v matmul run in bf16 (needed for
exp range). Softmax uses a shifted exp with per-row shift alpha[n] =
max(S[n, ::8]) + 45 (sampled row max + margin), injected as an extra
contraction channel (k row of ones, q row of -alpha) so exp(S - alpha)
reads straight out of PSUM; a row of ones appended to v^T makes the same
matmul accumulate sum(exp). The alpha shift cancels exactly in the
normalization, so its fp16 rounding is harmless.
"""

import sys

sys.path.insert(0, "/opt/trn_rl_repo")

import hashlib

import numpy as np

import concourse.bacc as bacc
import concourse.mybir as mybir
from concourse import tile

F16 = mybir.dt.float16
F32 = mybir.dt.float32
BF16 = mybir.dt.bfloat16
AF = mybir.ActivationFunctionType
ALU = mybir.AluOpType

EPS = 1e-5
ALPHA_MARGIN = 45.0
H = W = 64
N = H * W                 # 4096 positions (attention rows and keys)
MTILES = N // 128         # 32
NCORES = 4

WEIGHT_KEYS = [p + s for p in "qkvr" for s in ("w", "b", "g", "be", "m", "v")]


def _build_program(repeat=1):
    # repeat > 1 duplicates the whole per-call body (input DMAs included)
    # for differential hardware timing: wall(K) - wall(1) ~= (K-1) * HW time.
    nc = bacc.Bacc("TRN2", target_bir_lowering=False, debug=False)

    # xx: [f1 half0, f1 half1, f2 half0, f2 half1], each [128, 64, 64]
    xx_d = nc.dram_tensor("xx", [4, 128, 64, 64], F16, kind="ExternalInput")
    wq_d = nc.dram_tensor("wq", [128, 9, 2, 64], F16, kind="ExternalInput")
    wkv_d = nc.dram_tensor("wkv", [128, 9, 2, 128], F16, kind="ExternalInput")
    wr_d = nc.dram_tensor("wr", [64, 9, 256], F16, kind="ExternalInput")
    bn_d = nc.dram_tensor("bn", [128, 10], F32, kind="ExternalInput")
    out_d = nc.dram_tensor("out", [2, 128, 4096], F16, kind="ExternalOutput")

    with tile.TileContext(nc) as tc:
        with tc.tile_pool(name="per", bufs=1) as per, \
             tc.tile_pool(name="eb", bufs=4) as eb, \
             tc.tile_pool(name="sm", bufs=2) as sm, \
             tc.tile_pool(name="tp", bufs=3, space="PSUM") as tp, \
             tc.tile_pool(name="fp", bufs=4, space="PSUM") as fp:

            # ---- persistent SBUF tiles ----
            x1 = per.tile([128, 2, 66, 66], F16)
            x2 = per.tile([128, 2, 66, 66], F16)
            wq = per.tile([128, 9, 2, 64], F16)
            wkv = per.tile([128, 9, 2, 128], F16)
            wr = per.tile([64, 9, 256], F16)
            bn = per.tile([128, 10], F32)
            q_aug = per.tile([65, N], F16)
            k_aug = per.tile([65, N], F16)
            v_bf = per.tile([128, N], BF16)    # v lives at partitions 64..127
            vT = per.tile([128, MTILES, 80], BF16)  # 80: 32B-aligned stride for DMA-transpose dests
            f_pad = per.tile([64, 66, 66], F16)
            mcol = per.tile([128, 32], F32)
            nacol = per.tile([128, 32], F32)
            na_f32 = per.tile([1, N], F32)
            out_sb = per.tile([128, 2, 4096], F16)

            for rep in range(repeat):
              R = f"r{rep}_"
              nc.sync.dma_start(out=wkv[:, :, :, :], in_=wkv_d[:, :, :, :])
              nc.sync.dma_start(out=wq[:, :, :, :], in_=wq_d[:, :, :, :])
              nc.sync.dma_start(out=bn[:, :], in_=bn_d[:, :])

              # zero the padded borders, then land the raw features in the
              # interior; row slabs so the first conv tiles can start early
              nc.vector.memset(x1[:, :, :, :], 0.0)
              nc.vector.memset(x2[:, :, :, :], 0.0)
              for half in range(2):
                for r0, r1 in [(0, 18), (18, 34), (34, 49), (49, 64)]:
                    nc.sync.dma_start(out=x1[:, half, 1 + r0:1 + r1, 1:65],
                                      in_=xx_d[half, :, r0:r1, :])
              for half in range(2):
                for r0, r1 in [(0, 32), (32, 64)]:
                    nc.sync.dma_start(out=x2[:, half, 1 + r0:1 + r1, 1:65],
                                      in_=xx_d[2 + half, :, r0:r1, :])
              nc.sync.dma_start(out=wr[:, :, :], in_=wr_d[:, :, :])

              nc.vector.memset(k_aug[64:65, :], 1.0)
              nc.vector.memset(vT[:, :, 64:65], 1.0)
              nc.vector.memset(f_pad[:, :, :], 0.0)

              # ---- fused k+v conv (M=128: co 0..63 = k, 64..127 = v) ----
              for t in range(8):
                r0 = t * 8
                ps = tp.tile([128, 512], F32, name=f"{R}kv_{t}", tag="tpsum")
                for half in range(2):
                    for off in range(9):
                        dy, dx = off // 3, off % 3
                        nc.tensor.matmul(
                            ps[:, :], wkv[:, off, half, :],
                            x1[:, half, r0 + dy:r0 + dy + 8, dx:dx + W],
                            start=(half == 0 and off == 0),
                            stop=(half == 1 and off == 8))
                nc.scalar.activation(k_aug[0:64, r0 * W:(r0 + 8) * W], ps[0:64, :],
                                     AF.Relu, bias=bn[0:64, 3:4], scale=bn[0:64, 2:3])
                nc.scalar.activation(v_bf[64:128, r0 * W:(r0 + 8) * W], ps[64:128, :],
                                     AF.Relu, bias=bn[64:128, 3:4],
                                     scale=bn[64:128, 2:3])
                # v^T for this 512-col span (4 m-tiles) via DMA transpose
                for mt in range(t * 4, t * 4 + 4):
                    nc.sync.dma_start(out=vT[:, mt, 0:64],
                                      in_=v_bf[64:128, mt * 128:(mt + 1) * 128],
                                      transpose=True)

              # ---- q conv (M=64), interleaved with sampled row-max tiles ----
              for t in range(8):
                r0 = t * 8
                ps = tp.tile([128, 512], F32, name=f"{R}qc_{t}", tag="tpsum")
                for half in range(2):
                    for off in range(9):
                        dy, dx = off // 3, off % 3
                        nc.tensor.matmul(
                            ps[0:64, :], wq[:, off, half, :],
                            x2[:, half, r0 + dy:r0 + dy + 8, dx:dx + W],
                            start=(half == 0 and off == 0),
                            stop=(half == 1 and off == 8))
                nc.scalar.activation(q_aug[0:64, r0 * W:(r0 + 8) * W], ps[0:64, :],
                                     AF.Relu, bias=bn[0:64, 1:2], scale=bn[0:64, 0:1])
                # sampled row-max S_sub for the 4 fresh 128-col spans of q
                for st_ in range(t * 4, t * 4 + 4):
                    sps = tp.tile([128, 512], F32, name=f"{R}sub_{st_}", tag="tpsum")
                    nc.tensor.matmul(sps[:, :],
                                     q_aug[0:64, st_ * 128:(st_ + 1) * 128],
                                     k_aug[0:64, ::8], start=True, stop=True)
                    nc.vector.tensor_reduce(mcol[:, st_:st_ + 1], sps[:, :],
                                            axis=mybir.AxisListType.X, op=ALU.max)

              # -alpha = -(submax + MARGIN), spread to a [1, N] row
              nc.vector.tensor_scalar(nacol[:, :], mcol[:, :], -1.0, -ALPHA_MARGIN,
                                      ALU.mult, ALU.add)
              for t in range(32):
                nc.sync.dma_start(out=na_f32[:, t * 128:(t + 1) * 128],
                                  in_=nacol[:, t:t + 1])
              nc.vector.tensor_copy(q_aug[64:65, :], na_f32[:, :])

              # ---- attention: S^T -> exp -> attn @ v (+ sumexp row) ----
              # two row-groups of 2048, each split into 4 chunks of 512 cols;
              # 4 PSUM f-banks rotate between the groups
              for g in range(2):
                fbanks = [fp.tile([65, 512], F32, name=f"{R}fb_{g}_{c}",
                                  tag="fbank")
                          for c in range(4)]
                for m in range(MTILES):
                    for c in range(4):
                        n0 = g * 2048 + c * 512
                        st = tp.tile([128, 512], F32, name=f"{R}st_{g}_{m}_{c}",
                                     tag="tpsum")
                        nc.tensor.matmul(st[:, :], k_aug[:, m * 128:(m + 1) * 128],
                                         q_aug[:, n0:n0 + 512],
                                         start=True, stop=True)
                        e = eb.tile([128, 512], BF16, name=f"{R}e_{g}_{m}_{c}",
                                    tag="ebuf")
                        nc.scalar.activation(e[:, :], st[:, :], AF.Exp)
                        nc.tensor.matmul(fbanks[c][:, :], vT[:, m, 0:65], e[:, :],
                                         start=(m == 0), stop=(m == MTILES - 1))

                # normalize f and store into the padded conv-input layout
                for c in range(4):
                    n0 = g * 2048 + c * 512
                    row0 = n0 // W
                    rcp = sm.tile([1, 512], F32, name=f"{R}rcp{g}{c}", tag="rcp")
                    nc.vector.reciprocal(rcp[:, :], fbanks[c][64:65, :])
                    rb = sm.tile([64, 512], F32, name=f"{R}rb{g}{c}", tag="rb")
                    nc.gpsimd.partition_broadcast(rb[:, :], rcp[:, :])
                    nc.vector.tensor_tensor(
                        f_pad[:, 1 + row0:1 + row0 + 8, 1:65],
                        fbanks[c][0:64, :], rb[:, :], op=ALU.mult)

              # ---- final conv(64->256) + BN + ReLU (residual added on host) ----
              for coh in range(2):
                for t in range(8):
                    ps = tp.tile([128, 512], F32, name=f"{R}rps_{coh}_{t}",
                                 tag="tpsum")
                    for off in range(9):
                        dy, dx = off // 3, off % 3
                        nc.tensor.matmul(
                            ps[:, :], wr[:, off, coh * 128:(coh + 1) * 128],
                            f_pad[:, t * 8 + dy:t * 8 + dy + 8, dx:dx + W],
                            start=(off == 0), stop=(off == 8))
                    sc = bn[:, 6 + 2 * coh:7 + 2 * coh]
                    bi = bn[:, 7 + 2 * coh:8 + 2 * coh]
                    nc.scalar.activation(out_sb[:, coh, t * 512:(t + 1) * 512],
                                         ps[:, :], AF.Relu, bias=bi, scale=sc)
              for half in range(2):
                nc.sync.dma_start(out=out_d[half, :, :], in_=out_sb[:, half, :])

    nc.compile()
    return nc


class _Runtime:
    def __init__(self):
        import jax
        from jax.sharding import Mesh, NamedSharding, PartitionSpec
        from jax.experimental.shard_map import shard_map
        from concourse.bass2jax import (_bass_exec_p, install_neuronx_cc_hook,
                                        partition_id_tensor)

        self.jax = jax
        install_neuronx_cc_hook()
        nc = _build_program()
        self.nc = nc

        partition_name = (nc.partition_id_tensor.name
                          if nc.partition_id_tensor else None)
        in_names, out_names, out_avals = [], [], []
        for alloc in nc.m.functions[0].allocations:
            if not isinstance(alloc, mybir.MemoryLocationSet):
                continue
            name = alloc.memorylocations[0].name
            if alloc.kind == "ExternalInput":
                if name != partition_name:
                    in_names.append(name)
            elif alloc.kind == "ExternalOutput":
                out_names.append(name)
                out_avals.append(jax.core.ShapedArray(
                    tuple(alloc.tensor_shape), mybir.dt.np(alloc.dtype)))
        self.in_names = in_names
        n_in = len(in_names) + len(out_names)
        all_in_names = in_names + out_names + (
            [partition_name] if partition_name else [])

        def _body(*args):
            operands = list(args)
            if partition_name is not None:
                operands.append(partition_id_tensor())
            outs = _bass_exec_p.bind(
                *operands, out_avals=tuple(out_avals),
                in_names=tuple(all_in_names), out_names=tuple(out_names),
                lowering_input_output_aliases=(), sim_require_finite=True,
                sim_require_nnan=True, nc=nc)
            return tuple(outs)

        devices = jax.devices()[:NCORES]
        mesh = Mesh(np.asarray(devices), ("core",))
        self.sharding = NamedSharding(mesh, PartitionSpec("core"))
        self.fn = jax.jit(shard_map(
            _body, mesh=mesh, in_specs=(PartitionSpec("core"),) * n_in,
            out_specs=(PartitionSpec("core"),) * len(out_names),
            check_rep=False))

        # The NEFF writes every element of `out`, so the output operand only
        # has to exist — a persistent non-donated dummy avoids shipping
        # fresh zero buffers on every call.
        self.dummy_out = jax.device_put(
            np.zeros((NCORES * 2, 128, 4096), np.float16), self.sharding)

        # persistent pinned feature staging buffer
        self.xx_host = np.empty((NCORES * 4, 128, 64, 64), np.float16)

        self.weight_digest = None
        self.weight_dev = None

    def upload_weights(self, inputs):
        h = hashlib.blake2b(digest_size=16)
        arrs = {k: np.asarray(inputs[k], np.float32) for k in WEIGHT_KEYS}
        for k in WEIGHT_KEYS:
            h.update(np.ascontiguousarray(arrs[k]).tobytes())
        digest = h.digest()
        if digest == self.weight_digest:
            return
        # conv weights -> lhsT [ci, co] per (offset, ci_half)
        def lhsT(nm):
            w = arrs[nm]                                    # [64, 256, 3, 3]
            wt = w.transpose(2, 3, 1, 0).reshape(9, 2, 128, 64)
            return wt.transpose(2, 0, 1, 3)                 # [128, 9, 2, 64]
        wq = lhsT("qw").astype(np.float16)
        wkv = np.concatenate([lhsT("kw"), lhsT("vw")], axis=3).astype(np.float16)
        wr = np.ascontiguousarray(
            arrs["rw"].transpose(2, 3, 1, 0).reshape(9, 64, 256)
            .transpose(1, 0, 2)).astype(np.float16)         # [64, 9, 256]

        # bn cols: 0/1 = q scale/bias (parts 0..63); 2/3 = k (parts 0..63)
        # and v (parts 64..127) scale/bias; 6..9 = r conv halves.
        bnv = np.zeros((128, 10), np.float32)
        for p, rows, cols in [("q", slice(0, 64), (0, 1)),
                              ("k", slice(0, 64), (2, 3)),
                              ("v", slice(64, 128), (2, 3))]:
            inv = arrs[p + "g"] / np.sqrt(arrs[p + "v"] + EPS)
            bias = arrs[p + "b"] * inv + arrs[p + "be"] - arrs[p + "m"] * inv
            bnv[rows, cols[0]] = inv
            bnv[rows, cols[1]] = bias
        rinv = arrs["rg"] / np.sqrt(arrs["rv"] + EPS)
        rbias = arrs["rb"] * rinv + arrs["rbe"] - arrs["rm"] * rinv
        bnv[:, 6], bnv[:, 7] = rinv[0:128], rbias[0:128]
        bnv[:, 8], bnv[:, 9] = rinv[128:256], rbias[128:256]

        def rep(a):  # replicate a per-core array for all cores
            return np.ascontiguousarray(
                np.broadcast_to(a[None], (NCORES,) + a.shape)
            ).reshape((NCORES * a.shape[0],) + a.shape[1:])

        dev = {}
        for name, arr in [("wq", rep(wq)), ("wkv", rep(wkv)), ("wr", rep(wr)),
                          ("bn", rep(bnv))]:
            dev[name] = self.jax.device_put(arr, self.sharding)
        self.jax.block_until_ready(list(dev.values()))
        self.weight_dev = dev
        self.weight_digest = digest

    def __call__(self, inputs):
        self.upload_weights(inputs)
        f1 = np.asarray(inputs["feature1"], np.float32)
        f2 = np.asarray(inputs["feature2"], np.float32)
        # half-major staging: [b, {f1h0, f1h1, f2h0, f2h1}, 128, 64, 64];
        # contiguous assignment with inline fp32->fp16 cast
        xxv = self.xx_host.reshape(4, 4, 128, 64, 64)
        xxv[:, 0:2] = f1.reshape(4, 2, 128, 64, 64)
        xxv[:, 2:4] = f2.reshape(4, 2, 128, 64, 64)
        xx_dev = self.jax.device_put(self.xx_host, self.sharding)
        dev = {"xx": xx_dev, **self.weight_dev}
        outs = self.fn(*[dev[nm] for nm in self.in_names], self.dummy_out)
        out_np = np.asarray(outs[0])                 # [8, 128, 4096] fp16
        f = out_np.reshape(4, 2, 128, 64, 64)
        result = np.empty((4, 256, 64, 64), np.float32)
        np.add(f1.reshape(4, 2, 128, 64, 64), f,
               out=result.reshape(4, 2, 128, 64, 64))
        return result


_RT = None


def kernel(**inputs):
    global _RT
    if _RT is None:
        _RT = _Runtime()
    return _RT(inputs)


if __name__ == "__main__":
    rng = np.random.default_rng(0)
    ins = {}
    ins["feature1"] = rng.normal(size=(4, 256, 64, 64)).astype(np.float32)
    ins["feature2"] = rng.normal(size=(4, 256, 64, 64)).astype(np.float32)
    for p, cin, cout in [("q", 256, 64), ("k", 256, 64), ("v", 256, 64),
                         ("r", 64, 256)]:
        ins[p + "w"] = (rng.normal(size=(cout, cin, 3, 3)) * 0.05).astype(np.float32)
        ins[p + "b"] = np.zeros(cout, np.float32)
        ins[p + "g"] = np.ones(cout, np.float32)
        ins[p + "be"] = np.zeros(cout, np.float32)
        ins[p + "m"] = np.zeros(cout, np.float32)
        ins[p + "v"] = np.ones(cout, np.float32)
    out = kernel(**ins)
    print("ran", out.shape, out.dtype, np.abs(out).mean())


# revision 8
# speedup vs baseline: 7.9923x; 1.2038x over previous
"""Trainium2 Bass kernel for CrossModalityPositionAttention.

Model (per batch element b of 4):
  q = ConvBNReLU(feature2[b]; qw)   [64, 64, 64]
  k = ConvBNReLU(feature1[b]; kw)
  v = ConvBNReLU(feature1[b]; vw)
  attn = softmax(q^T k over channels), f = v @ attn^T
  out = feature1[b] + ConvBNReLU(f; rw)   [256, 64, 64]

Sharding: 4 cores, one full batch element per core (cores 4..7 idle). The
per-call wall clock is dominated by the host<->device axon link (~10ms/MB
up, ~25ms/MB down, ~75ms fixed per transfer/launch), not by device
compute (tens of microseconds of PE work per core), so the split
minimizes link bytes in both directions:

  - up: one fp16 upload holding exactly one copy of each feature map
    (16.8MB total, the information floor at fp16), in half-major
    [2,128,64,64] dram layout so host staging is a contiguous
    memcpy-with-cast (no transpose, no padding bytes — the device pads
    via memset + interior DMA);
  - down: only the normalized 64-channel attention output f ([64,4096]
    fp16, 2.1MB total). The cheap final conv (64->256, 2.4 GFLOP total)
    plus BN/ReLU/residual runs on the host in fp32, pipelined under the
    per-shard fetches — this quarters the download vs shipping the
    256-channel conv output;
  - all weight-derived buffers live on device, re-uploaded only when the
    weight bytes change (blake2b check);
  - a single jitted shard_map executable is built once and reused.

Numerics: fp16 features/weights (10-bit mantissa, ~5e-4 rel — the
near-one-hot softmax needs q/k logits accurate to ~0.1 absolute, which
bf16's 8-bit mantissa would miss), fp32 PSUM accumulation everywhere;
attention probabilities and the attn@v matmul run in bf16 (needed for
exp range). Softmax uses a shifted exp with per-row shift alpha[n] =
max(S[n, ::8]) + 45 (sampled row max + margin), injected as an extra
contraction channel (k row of ones, q row of -alpha) so exp(S - alpha)
reads straight out of PSUM; a row of ones appended to v^T makes the same
matmul accumulate sum(exp). The alpha shift cancels exactly in the
normalization, so its fp16 rounding is harmless.
"""

import sys

sys.path.insert(0, "/opt/trn_rl_repo")

import hashlib
from concurrent.futures import ThreadPoolExecutor

import numpy as np

import concourse.bacc as bacc
import concourse.mybir as mybir
from concourse import tile

F16 = mybir.dt.float16
F32 = mybir.dt.float32
BF16 = mybir.dt.bfloat16
AF = mybir.ActivationFunctionType
ALU = mybir.AluOpType

EPS = 1e-5
ALPHA_MARGIN = 45.0
H = W = 64
N = H * W                 # 4096 positions (attention rows and keys)
MTILES = N // 128         # 32
NCORES = 4

WEIGHT_KEYS = [p + s for p in "qkvr" for s in ("w", "b", "g", "be", "m", "v")]


def _build_program(repeat=1):
    # repeat > 1 duplicates the whole per-call body (input DMAs included)
    # for differential hardware timing: wall(K) - wall(1) ~= (K-1) * HW time.
    nc = bacc.Bacc("TRN2", target_bir_lowering=False, debug=False)

    # xx: [f1 half0, f1 half1, f2 half0, f2 half1], each [128, 64, 64]
    xx_d = nc.dram_tensor("xx", [4, 128, 64, 64], F16, kind="ExternalInput")
    wq_d = nc.dram_tensor("wq", [128, 9, 2, 64], F16, kind="ExternalInput")
    wkv_d = nc.dram_tensor("wkv", [128, 9, 2, 128], F16, kind="ExternalInput")
    bn_d = nc.dram_tensor("bn", [128, 4], F32, kind="ExternalInput")
    out_d = nc.dram_tensor("out", [64, N], F16, kind="ExternalOutput")

    with tile.TileContext(nc) as tc:
        with tc.tile_pool(name="per", bufs=1) as per, \
             tc.tile_pool(name="eb", bufs=4) as eb, \
             tc.tile_pool(name="sm", bufs=2) as sm, \
             tc.tile_pool(name="tp", bufs=3, space="PSUM") as tp, \
             tc.tile_pool(name="fp", bufs=4, space="PSUM") as fp:

            # ---- persistent SBUF tiles ----
            x1 = per.tile([128, 2, 66, 66], F16)
            x2 = per.tile([128, 2, 66, 66], F16)
            wq = per.tile([128, 9, 2, 64], F16)
            wkv = per.tile([128, 9, 2, 128], F16)
            bn = per.tile([128, 4], F32)
            q_aug = per.tile([65, N], F16)
            k_aug = per.tile([65, N], F16)
            v_bf = per.tile([128, N], BF16)    # v lives at partitions 64..127
            vT = per.tile([128, MTILES, 80], BF16)  # 80: 32B-aligned stride for DMA-transpose dests
            mcol = per.tile([128, 32], F32)
            nacol = per.tile([128, 32], F32)
            na_f32 = per.tile([1, N], F32)
            out_sb = per.tile([64, N], F16)

            for rep in range(repeat):
              R = f"r{rep}_"
              nc.sync.dma_start(out=wkv[:, :, :, :], in_=wkv_d[:, :, :, :])
              nc.sync.dma_start(out=wq[:, :, :, :], in_=wq_d[:, :, :, :])
              nc.sync.dma_start(out=bn[:, :], in_=bn_d[:, :])

              # zero the padded borders, then land the raw features in the
              # interior; row slabs so the first conv tiles can start early
              nc.vector.memset(x1[:, :, :, :], 0.0)
              nc.vector.memset(x2[:, :, :, :], 0.0)
              for half in range(2):
                for r0, r1 in [(0, 18), (18, 34), (34, 49), (49, 64)]:
                    nc.sync.dma_start(out=x1[:, half, 1 + r0:1 + r1, 1:65],
                                      in_=xx_d[half, :, r0:r1, :])
              for half in range(2):
                for r0, r1 in [(0, 32), (32, 64)]:
                    nc.sync.dma_start(out=x2[:, half, 1 + r0:1 + r1, 1:65],
                                      in_=xx_d[2 + half, :, r0:r1, :])

              nc.vector.memset(k_aug[64:65, :], 1.0)
              nc.vector.memset(vT[:, :, 64:65], 1.0)

              # ---- fused k+v conv (M=128: co 0..63 = k, 64..127 = v) ----
              for t in range(8):
                r0 = t * 8
                ps = tp.tile([128, 512], F32, name=f"{R}kv_{t}", tag="tpsum")
                for half in range(2):
                    for off in range(9):
                        dy, dx = off // 3, off % 3
                        nc.tensor.matmul(
                            ps[:, :], wkv[:, off, half, :],
                            x1[:, half, r0 + dy:r0 + dy + 8, dx:dx + W],
                            start=(half == 0 and off == 0),
                            stop=(half == 1 and off == 8))
                nc.scalar.activation(k_aug[0:64, r0 * W:(r0 + 8) * W], ps[0:64, :],
                                     AF.Relu, bias=bn[0:64, 3:4], scale=bn[0:64, 2:3])
                nc.scalar.activation(v_bf[64:128, r0 * W:(r0 + 8) * W], ps[64:128, :],
                                     AF.Relu, bias=bn[64:128, 3:4],
                                     scale=bn[64:128, 2:3])
                # v^T for this 512-col span (4 m-tiles) via DMA transpose
                for mt in range(t * 4, t * 4 + 4):
                    nc.sync.dma_start(out=vT[:, mt, 0:64],
                                      in_=v_bf[64:128, mt * 128:(mt + 1) * 128],
                                      transpose=True)

              # ---- q conv (M=64), interleaved with sampled row-max tiles ----
              for t in range(8):
                r0 = t * 8
                ps = tp.tile([128, 512], F32, name=f"{R}qc_{t}", tag="tpsum")
                for half in range(2):
                    for off in range(9):
                        dy, dx = off // 3, off % 3
                        nc.tensor.matmul(
                            ps[0:64, :], wq[:, off, half, :],
                            x2[:, half, r0 + dy:r0 + dy + 8, dx:dx + W],
                            start=(half == 0 and off == 0),
                            stop=(half == 1 and off == 8))
                nc.scalar.activation(q_aug[0:64, r0 * W:(r0 + 8) * W], ps[0:64, :],
                                     AF.Relu, bias=bn[0:64, 1:2], scale=bn[0:64, 0:1])
                # sampled row-max S_sub for the 4 fresh 128-col spans of q
                for st_ in range(t * 4, t * 4 + 4):
                    sps = tp.tile([128, 512], F32, name=f"{R}sub_{st_}", tag="tpsum")
                    nc.tensor.matmul(sps[:, :],
                                     q_aug[0:64, st_ * 128:(st_ + 1) * 128],
                                     k_aug[0:64, ::8], start=True, stop=True)
                    nc.vector.tensor_reduce(mcol[:, st_:st_ + 1], sps[:, :],
                                            axis=mybir.AxisListType.X, op=ALU.max)

              # -alpha = -(submax + MARGIN), spread to a [1, N] row
              nc.vector.tensor_scalar(nacol[:, :], mcol[:, :], -1.0, -ALPHA_MARGIN,
                                      ALU.mult, ALU.add)
              for t in range(32):
                nc.sync.dma_start(out=na_f32[:, t * 128:(t + 1) * 128],
                                  in_=nacol[:, t:t + 1])
              nc.vector.tensor_copy(q_aug[64:65, :], na_f32[:, :])

              # ---- attention: S^T -> exp -> attn @ v (+ sumexp row) ----
              # two row-groups of 2048, each split into 4 chunks of 512 cols;
              # 4 PSUM f-banks rotate between the groups
              for g in range(2):
                fbanks = [fp.tile([65, 512], F32, name=f"{R}fb_{g}_{c}",
                                  tag="fbank")
                          for c in range(4)]
                for m in range(MTILES):
                    for c in range(4):
                        n0 = g * 2048 + c * 512
                        st = tp.tile([128, 512], F32, name=f"{R}st_{g}_{m}_{c}",
                                     tag="tpsum")
                        nc.tensor.matmul(st[:, :], k_aug[:, m * 128:(m + 1) * 128],
                                         q_aug[:, n0:n0 + 512],
                                         start=True, stop=True)
                        e = eb.tile([128, 512], BF16, name=f"{R}e_{g}_{m}_{c}",
                                    tag="ebuf")
                        nc.scalar.activation(e[:, :], st[:, :], AF.Exp)
                        nc.tensor.matmul(fbanks[c][:, :], vT[:, m, 0:65], e[:, :],
                                         start=(m == 0), stop=(m == MTILES - 1))

                # normalize f: divide by the sum-exp row, store fp16
                for c in range(4):
                    n0 = g * 2048 + c * 512
                    rcp = sm.tile([1, 512], F32, name=f"{R}rcp{g}{c}", tag="rcp")
                    nc.vector.reciprocal(rcp[:, :], fbanks[c][64:65, :])
                    rb = sm.tile([64, 512], F32, name=f"{R}rb{g}{c}", tag="rb")
                    nc.gpsimd.partition_broadcast(rb[:, :], rcp[:, :])
                    nc.vector.tensor_tensor(out_sb[:, n0:n0 + 512],
                                            fbanks[c][0:64, :], rb[:, :],
                                            op=ALU.mult)

              nc.sync.dma_start(out=out_d[:, :], in_=out_sb[:, :])

    nc.compile()
    return nc


class _Runtime:
    def __init__(self):
        import jax
        from jax.sharding import Mesh, NamedSharding, PartitionSpec
        from jax.experimental.shard_map import shard_map
        from concourse.bass2jax import (_bass_exec_p, install_neuronx_cc_hook,
                                        partition_id_tensor)

        self.jax = jax
        install_neuronx_cc_hook()
        nc = _build_program()
        self.nc = nc

        partition_name = (nc.partition_id_tensor.name
                          if nc.partition_id_tensor else None)
        in_names, out_names, out_avals = [], [], []
        for alloc in nc.m.functions[0].allocations:
            if not isinstance(alloc, mybir.MemoryLocationSet):
                continue
            name = alloc.memorylocations[0].name
            if alloc.kind == "ExternalInput":
                if name != partition_name:
                    in_names.append(name)
            elif alloc.kind == "ExternalOutput":
                out_names.append(name)
                out_avals.append(jax.core.ShapedArray(
                    tuple(alloc.tensor_shape), mybir.dt.np(alloc.dtype)))
        self.in_names = in_names
        n_in = len(in_names) + len(out_names)
        all_in_names = in_names + out_names + (
            [partition_name] if partition_name else [])

        def _body(*args):
            operands = list(args)
            if partition_name is not None:
                operands.append(partition_id_tensor())
            outs = _bass_exec_p.bind(
                *operands, out_avals=tuple(out_avals),
                in_names=tuple(all_in_names), out_names=tuple(out_names),
                lowering_input_output_aliases=(), sim_require_finite=True,
                sim_require_nnan=True, nc=nc)
            return tuple(outs)

        devices = jax.devices()[:NCORES]
        mesh = Mesh(np.asarray(devices), ("core",))
        self.sharding = NamedSharding(mesh, PartitionSpec("core"))
        self.fn = jax.jit(shard_map(
            _body, mesh=mesh, in_specs=(PartitionSpec("core"),) * n_in,
            out_specs=(PartitionSpec("core"),) * len(out_names),
            check_rep=False))

        # The NEFF writes every element of `out`, so the output operand only
        # has to exist — a persistent non-donated dummy avoids shipping
        # fresh zero buffers on every call.
        self.dummy_out = jax.device_put(
            np.zeros((NCORES * 64, N), np.float16), self.sharding)

        # persistent pinned feature staging buffer
        self.xx_host = np.empty((NCORES * 4, 128, 64, 64), np.float16)
        self.fpad = np.zeros((64, 66, 66), np.float32)   # host conv scratch
        self.pool = ThreadPoolExecutor(NCORES)

        self.weight_digest = None
        self.weight_dev = None
        self.host_w = None

    def upload_weights(self, inputs):
        h = hashlib.blake2b(digest_size=16)
        arrs = {k: np.asarray(inputs[k], np.float32) for k in WEIGHT_KEYS}
        for k in WEIGHT_KEYS:
            h.update(np.ascontiguousarray(arrs[k]).tobytes())
        digest = h.digest()
        if digest == self.weight_digest:
            return
        # conv weights -> lhsT [ci, co] per (offset, ci_half)
        def lhsT(nm):
            w = arrs[nm]                                    # [64, 256, 3, 3]
            wt = w.transpose(2, 3, 1, 0).reshape(9, 2, 128, 64)
            return wt.transpose(2, 0, 1, 3)                 # [128, 9, 2, 64]
        wq = lhsT("qw").astype(np.float16)
        wkv = np.concatenate([lhsT("kw"), lhsT("vw")], axis=3).astype(np.float16)

        # bn cols: 0/1 = q scale/bias (parts 0..63); 2/3 = k (parts 0..63)
        # and v (parts 64..127) scale/bias
        bnv = np.zeros((128, 4), np.float32)
        for p, rows, cols in [("q", slice(0, 64), (0, 1)),
                              ("k", slice(0, 64), (2, 3)),
                              ("v", slice(64, 128), (2, 3))]:
            inv = arrs[p + "g"] / np.sqrt(arrs[p + "v"] + EPS)
            bias = arrs[p + "b"] * inv + arrs[p + "be"] - arrs[p + "m"] * inv
            bnv[rows, cols[0]] = inv
            bnv[rows, cols[1]] = bias

        # host-side final conv: W [256, 576] with BN scale folded in;
        # column order (ci, ky, kx) matches the as_strided im2col below
        rinv = arrs["rg"] / np.sqrt(arrs["rv"] + EPS)
        rbias = (arrs["rb"] * rinv + arrs["rbe"] - arrs["rm"] * rinv)
        wm = arrs["rw"].reshape(256, 576) * rinv[:, None]
        self.host_w = (np.ascontiguousarray(wm), rbias[:, None].copy())

        def rep(a):  # replicate a per-core array for all cores
            return np.ascontiguousarray(
                np.broadcast_to(a[None], (NCORES,) + a.shape)
            ).reshape((NCORES * a.shape[0],) + a.shape[1:])

        dev = {}
        for name, arr in [("wq", rep(wq)), ("wkv", rep(wkv)), ("bn", rep(bnv))]:
            dev[name] = self.jax.device_put(arr, self.sharding)
        self.jax.block_until_ready(list(dev.values()))
        self.weight_dev = dev
        self.weight_digest = digest

    def __call__(self, inputs):
        self.upload_weights(inputs)
        f1 = np.asarray(inputs["feature1"], np.float32)
        f2 = np.asarray(inputs["feature2"], np.float32)
        # half-major staging: [b, {f1h0, f1h1, f2h0, f2h1}, 128, 64, 64];
        # contiguous assignment with inline fp32->fp16 cast
        xxv = self.xx_host.reshape(4, 4, 128, 64, 64)
        xxv[:, 0:2] = f1.reshape(4, 2, 128, 64, 64)
        xxv[:, 2:4] = f2.reshape(4, 2, 128, 64, 64)
        xx_dev = self.jax.device_put(self.xx_host, self.sharding)
        dev = {"xx": xx_dev, **self.weight_dev}
        outs = self.fn(*[dev[nm] for nm in self.in_names], self.dummy_out)

        # fetch per-batch shards in the background; run the final conv
        # (64->256, fp32) + BN + ReLU + residual on the host while later
        # shards stream down
        shards = sorted(outs[0].addressable_shards,
                        key=lambda s: s.index[0].start or 0)
        futures = [self.pool.submit(lambda s=s: np.asarray(s.data))
                   for s in shards]
        wm, rbias = self.host_w
        result = np.empty((4, 256, 64, 64), np.float32)
        fpad = self.fpad
        for b in range(4):
            fb = futures[b].result()                    # [64, 4096] fp16
            fpad[:, 1:65, 1:65] = fb.reshape(64, 64, 64)
            cols = np.lib.stride_tricks.as_strided(
                fpad, shape=(64, 3, 3, 64, 64),
                strides=(fpad.strides[0], fpad.strides[1], fpad.strides[2],
                         fpad.strides[1], fpad.strides[2]))
            c = wm @ cols.reshape(576, 4096)
            c += rbias
            np.maximum(c, 0.0, out=c)
            c += f1[b].reshape(256, 4096)
            result[b] = c.reshape(256, 64, 64)
        return result


_RT = None


def kernel(**inputs):
    global _RT
    if _RT is None:
        _RT = _Runtime()
    return _RT(inputs)


if __name__ == "__main__":
    rng = np.random.default_rng(0)
    ins = {}
    ins["feature1"] = rng.normal(size=(4, 256, 64, 64)).astype(np.float32)
    ins["feature2"] = rng.normal(size=(4, 256, 64, 64)).astype(np.float32)
    for p, cin, cout in [("q", 256, 64), ("k", 256, 64), ("v", 256, 64),
                         ("r", 64, 256)]:
        ins[p + "w"] = (rng.normal(size=(cout, cin, 3, 3)) * 0.05).astype(np.float32)
        ins[p + "b"] = np.zeros(cout, np.float32)
        ins[p + "g"] = np.ones(cout, np.float32)
        ins[p + "be"] = np.zeros(cout, np.float32)
        ins[p + "m"] = np.zeros(cout, np.float32)
        ins[p + "v"] = np.ones(cout, np.float32)
    out = kernel(**ins)
    print("ran", out.shape, out.dtype, np.abs(out).mean())
